# revision 37
# baseline (speedup 1.0000x reference)
"""Trainium2 Bass kernel for nn_DecoderCell (GRU-style decoder cell).

Reference computation (per batch row):
    r      = sigmoid(x @ Wr.T + hprev @ Ur.T + c @ Cr.T + br)
    z      = sigmoid(x @ Wz.T + hprev @ Uz.T + c @ Cz.T + bz)
    h_prop = tanh  (x @ Wh.T + (r * hprev) @ Uh.T + c @ Ch.T + bh)
    out    = z * h_prop + (1 - z) * hprev

Shapes: B=8192, IN=1024, H=1024, c is [B, 2H].

Strategy:
  - Data-parallel across 8 NeuronCores: batch shard of 1024 rows per core,
    weights replicated. No collectives.
  - All compute in the "transposed domain": per core we hold
    AT = [x | hprev | c].T  -> [4096, 1024]  (k-major: the contraction dim
    lives on SBUF partitions) and per-gate weights M_g = [W_g|U_g|C_g].T
    -> [4096, 1024].  Gate pre-activations come out as [H, B_s] tiles, so
    the r*hprev product needed by the Uh matmul is produced directly in
    k-major layout and no on-device transposes are required.
  - Matmul operands are fp16 (converted on host): 10-bit mantissa keeps
    accuracy near TF32 for this data (everything is O(1), no range risk)
    while halving HBM traffic; accumulation is fp32 in PSUM.  Measured
    end-to-end RMS relative error ~3.7e-4.
  - PE loop: per gate, 4 column-quarter groups of 4 PSUM banks each
    (2 j-tiles x 2 batch-slices of 512), accumulating over 32 k-tiles of
    128.  The 8-bank PSUM pool double-buffers groups; weight slabs stream
    from HBM through a prefetch pool.  The PE runs at ~95% occupancy;
    kernel time is within ~10% of the 128x128@2.4GHz matmul roofline.
  - Biases are fused into the sigmoid/tanh activation instruction
    (activation computes func(in*scale + bias) with a per-partition bias).
  - A post-schedule BIR pass removes back-to-back identical LDWEIGHTS:
    Tile emits one per matmul even when consecutive matmuls reuse the
    same stationary tile (our two 512-wide batch slices), and the PE
    array keeps its weights across matmuls, so the repeats are no-ops.
"""

import sys

sys.path.insert(0, "/opt/trn_rl_repo")

import numpy as np
from contextlib import ExitStack

B = 8192
IN = 1024
H = 1024
NCORES = 8
BS = B // NCORES          # batch rows per core
KT = 32                   # 128-row k-tiles in the 4096-deep contraction
NB = BS // 512            # 512-wide moving slices per core
KSLAB = 4                 # k-tiles per weight-slab DMA

_CACHE = {}


def _dedup_ldweights(nc, mybir):
    """Drop redundant InstLdweights from the scheduled BIR.

    Tile splits every non-fp32 matmul into an explicit LDWEIGHTS + MATMUL
    pair, and the walrus invocation runs with --enable-ldw-opt=false, so
    back-to-back matmuls that reuse the same stationary tile each pay a
    full weight reload. The PE array keeps its weights across matmuls, so
    an LDWEIGHTS identical to the immediately preceding one (and carrying
    no semaphore waits or updates) is a no-op — remove it.
    """
    removed = 0
    for f in nc.m.functions:
        for bb in f.blocks:
            keep = []
            prev_sig = None
            for inst in bb.instructions:
                tn = type(inst).__name__
                if getattr(inst, "engine", None) == mybir.EngineType.PE:
                    if tn == "InstLdweights":
                        sig = str(inst.ins[0]) if inst.ins else None
                        si = inst.sync_info
                        clean = si is None or (
                            len(si.on_wait) == 0 and len(si.on_update) == 0
                        )
                        if sig is not None and sig == prev_sig and clean:
                            removed += 1
                            continue
                        prev_sig = sig
                    elif tn in ("InstMatmult", "InstEventSemaphore", "InstNoOp"):
                        pass  # these don't disturb the loaded weights
                    else:
                        prev_sig = None
                keep.append(inst)
            bb.instructions[:] = keep
    return removed


def _build_nc():
    import concourse.bacc as bacc
    import concourse.tile as tile
    from concourse import mybir

    f32 = mybir.dt.float32
    f16 = mybir.dt.float16
    SIG = mybir.ActivationFunctionType.Sigmoid
    TANH = mybir.ActivationFunctionType.Tanh

    nc = bacc.Bacc("TRN2", target_bir_lowering=False, debug=False)

    at_d = nc.dram_tensor("at", [4 * IN, BS], f16, kind="ExternalInput")
    w_d = {
        g: nc.dram_tensor(f"w{g}", [4 * IN, H], f16, kind="ExternalInput")
        for g in "rzh"
    }
    b_d = {
        g: nc.dram_tensor(f"b{g}", [128, 8], f32, kind="ExternalInput")
        for g in "rzh"
    }
    out_d = nc.dram_tensor("out_t", [H, BS], f32, kind="ExternalOutput")

    with tile.TileContext(nc) as tc:
        with ExitStack() as ctx:
            pp = ctx.enter_context(tc.tile_pool(name="persist", bufs=1))
            wp = ctx.enter_context(tc.tile_pool(name="wslab", bufs=12))
            rp = ctx.enter_context(tc.tile_pool(name="rtmp", bufs=4))
            hpp = ctx.enter_context(tc.tile_pool(name="hprop", bufs=4))
            op = ctx.enter_context(tc.tile_pool(name="otile", bufs=4))
            psp = ctx.enter_context(tc.tile_pool(name="ps", bufs=8, space="PSUM"))

            at_t = [pp.tile([128, BS], f16, tag=f"at{k}", name=f"at{k}") for k in range(KT)]
            rh_t = [pp.tile([128, BS], f16, tag=f"rh{j}", name=f"rh{j}") for j in range(8)]
            z_t = [
                [pp.tile([128, 512], f16, tag=f"z{j}_{b}", name=f"z{j}_{b}") for b in range(NB)]
                for j in range(8)
            ]
            bias_t = {g: pp.tile([128, 8], f32, tag=f"bias{g}", name=f"bias{g}") for g in "rzh"}

            at_dma = [None] * KT

            def ensure_at(k):
                if at_dma[k] is None:
                    at_dma[k] = nc.sync.dma_start(
                        at_t[k][:], at_d.ap()[k * 128:(k + 1) * 128, :]
                    )
                return at_dma[k]

            bias_loaded = [False]

            def ensure_bias():
                # biases are first needed at the first epilogue (~30us in);
                # keep them out of the critical DMA prefix
                if not bias_loaded[0]:
                    for g in "rzh":
                        nc.sync.dma_start(bias_t[g][:], b_d[g].ap()[:, :])
                    bias_loaded[0] = True

            def moving(g, k, b):
                # h-gate contracts k-tiles 8..15 against r*hprev, not hprev
                if g == "h" and 8 <= k < 16:
                    return rh_t[k - 8][:, b * 512:(b + 1) * 512]
                return at_t[k][:, b * 512:(b + 1) * 512]

            def do_group(g, jq):
                # output tiles: j in [jq*256, (jq+1)*256), all BS batch cols
                ps = {}
                for jl in range(2):
                    for b in range(NB):
                        ps[(jl, b)] = psp.tile([128, 512], f32, tag="ps",
                                               name=f"ps_{g}_{jq}_{jl}_{b}")
                for ks in range(KT // KSLAB):
                    k0 = KSLAB * ks
                    slab = wp.tile([128, KSLAB, 256], f16, tag="w", name=f"w_{g}_{jq}_{ks}")
                    src = w_d[g].ap()[k0 * 128:(k0 + KSLAB) * 128,
                                      jq * 256:(jq + 1) * 256]
                    nc.sync.dma_start(slab[:], src.rearrange("(a p) j -> p a j", p=128))
                    for dk in range(KSLAB):
                        k = k0 + dk
                        if not (g == "h" and 8 <= k < 16):
                            ensure_at(k)
                        for jl in range(2):
                            lhsT = slab[:, dk, jl * 128:(jl + 1) * 128]
                            for b in range(NB):
                                nc.tensor.matmul(
                                    ps[(jl, b)][:],
                                    lhsT,
                                    moving(g, k, b),
                                    start=(k == 0),
                                    stop=(k == KT - 1),
                                )
                ensure_bias()
                for jl in range(2):
                    jt = 2 * jq + jl
                    for b in range(NB):
                        pst = ps[(jl, b)]
                        bias_ap = bias_t[g][:, jt:jt + 1]
                        bsl = slice(b * 512, (b + 1) * 512)
                        if g == "r":
                            tmp = rp.tile([128, 512], f32, tag="rt", name=f"rt_{jt}_{b}")
                            nc.scalar.activation(tmp[:], pst[:], SIG, bias=bias_ap)
                            nc.vector.tensor_mul(
                                rh_t[jt][:, bsl], tmp[:], at_t[8 + jt][:, bsl]
                            )
                        elif g == "z":
                            nc.scalar.activation(z_t[jt][b][:], pst[:], SIG, bias=bias_ap)
                        else:
                            hp = hpp.tile([128, 512], f32, tag="hp", name=f"hp_{jt}_{b}")
                            nc.scalar.activation(hp[:], pst[:], TANH, bias=bias_ap)
                            hT = at_t[8 + jt][:, bsl]
                            ot = op.tile([128, 512], f32, tag="ot", name=f"ot_{jt}_{b}")
                            # out = h + z*(hp - h)
                            nc.vector.tensor_sub(ot[:], hp[:], hT)
                            nc.vector.tensor_mul(hp[:], z_t[jt][b][:], ot[:])
                            nc.vector.tensor_add(ot[:], hp[:], hT)
                            nc.sync.dma_start(
                                out_d.ap()[jt * 128:(jt + 1) * 128, bsl], ot[:]
                            )

            for g in ("r", "z", "h"):
                for jq in range(4):
                    do_group(g, jq)

    _dedup_ldweights(nc, mybir)
    nc.finalize()
    return nc


def _get_nc():
    if "nc" not in _CACHE:
        _CACHE["nc"] = _build_nc()
    return _CACHE["nc"]


def _host_prep(inputs):
    x = np.asarray(inputs["x"], dtype=np.float32)
    hprev = np.asarray(inputs["hprev"], dtype=np.float32)
    c = np.asarray(inputs["c"], dtype=np.float32)
    A = np.concatenate([x, hprev, c], axis=1)                  # [B, 4096]
    AT = np.ascontiguousarray(A.T.astype(np.float16))          # [4096, B]
    w = {}
    for g, (W, U, C) in {
        "r": (inputs["Wr"], inputs["Ur"], inputs["Cr"]),
        "z": (inputs["Wz"], inputs["Uz"], inputs["Cz"]),
        "h": (inputs["Wh"], inputs["Uh"], inputs["Ch"]),
    }.items():
        M = np.concatenate(
            [np.asarray(W, np.float32), np.asarray(U, np.float32),
             np.asarray(C, np.float32)],
            axis=1,
        )
        w[g] = np.ascontiguousarray(M.T.astype(np.float16))    # [4096, 1024]
    bias = {
        g: np.ascontiguousarray(
            np.asarray(inputs["b" + g], dtype=np.float32).reshape(8, 128).T
        )
        for g in "rzh"
    }
    return AT, w, bias


def _in_maps(inputs):
    AT, w, bias = _host_prep(inputs)
    maps = []
    for s in range(NCORES):
        maps.append(
            {
                "at": np.ascontiguousarray(AT[:, s * BS:(s + 1) * BS]),
                "wr": w["r"],
                "wz": w["z"],
                "wh": w["h"],
                "br": bias["r"],
                "bz": bias["z"],
                "bh": bias["h"],
            }
        )
    return maps


def run_device(inputs, trace=False, **kwargs):
    """Run the SPMD kernel; returns (full_output, BassKernelResults)."""
    from concourse.bass_utils import run_bass_kernel_spmd

    nc = _get_nc()
    res = run_bass_kernel_spmd(
        nc, _in_maps(inputs), core_ids=list(range(NCORES)), trace=trace, **kwargs
    )
    out = np.empty((B, H), dtype=np.float32)
    for s in range(NCORES):
        out[s * BS:(s + 1) * BS, :] = res.results[s]["out_t"].T
    return out, res


def kernel(**inputs):
    out, _ = run_device(inputs, trace=False)
    return out


# revision 38
# speedup vs baseline: 1.0106x; 1.0106x over previous
"""Trainium2 Bass kernel for nn_DecoderCell (GRU-style decoder cell).

Reference computation (per batch row):
    r      = sigmoid(x @ Wr.T + hprev @ Ur.T + c @ Cr.T + br)
    z      = sigmoid(x @ Wz.T + hprev @ Uz.T + c @ Cz.T + bz)
    h_prop = tanh  (x @ Wh.T + (r * hprev) @ Uh.T + c @ Ch.T + bh)
    out    = z * h_prop + (1 - z) * hprev

Shapes: B=8192, IN=1024, H=1024, c is [B, 2H].

Strategy:
  - Data-parallel across 8 NeuronCores: batch shard of 1024 rows per core,
    weights replicated. No collectives.
  - All compute in the "transposed domain": per core we hold
    AT = [x | hprev | c].T  -> [4096, 1024]  (k-major: the contraction dim
    lives on SBUF partitions) and per-gate weights M_g = [W_g|U_g|C_g].T
    -> [4096, 1024].  Gate pre-activations come out as [H, B_s] tiles, so
    the r*hprev product needed by the Uh matmul is produced directly in
    k-major layout and no on-device transposes are required.
  - Matmul operands are fp16 (converted on host): 10-bit mantissa keeps
    accuracy near TF32 for this data (everything is O(1), no range risk)
    while halving HBM traffic; accumulation is fp32 in PSUM.  Measured
    end-to-end RMS relative error ~3.7e-4.
  - PE loop: per gate, 4 column-quarter groups of 4 PSUM banks each
    (2 j-tiles x 2 batch-slices of 512), accumulating over 32 k-tiles of
    128.  The 8-bank PSUM pool double-buffers groups; weight slabs stream
    from HBM through a prefetch pool.  The PE runs at ~95% occupancy;
    kernel time is within ~10% of the 128x128@2.4GHz matmul roofline.
  - Biases are fused into the sigmoid/tanh activation instruction
    (activation computes func(in*scale + bias) with a per-partition bias).
  - A post-schedule BIR pass removes back-to-back identical LDWEIGHTS:
    Tile emits one per matmul even when consecutive matmuls reuse the
    same stationary tile (our two 512-wide batch slices), and the PE
    array keeps its weights across matmuls, so the repeats are no-ops.
"""

import sys

sys.path.insert(0, "/opt/trn_rl_repo")

import numpy as np
from contextlib import ExitStack

B = 8192
IN = 1024
H = 1024
NCORES = 8
BS = B // NCORES          # batch rows per core
KT = 32                   # 128-row k-tiles in the 4096-deep contraction
NB = BS // 512            # 512-wide moving slices per core
KSLAB = 4                 # k-tiles per weight-slab DMA

_CACHE = {}


def _dedup_ldweights(nc, mybir):
    """Drop redundant InstLdweights from the scheduled BIR.

    Tile splits every non-fp32 matmul into an explicit LDWEIGHTS + MATMUL
    pair, and the walrus invocation runs with --enable-ldw-opt=false, so
    back-to-back matmuls that reuse the same stationary tile each pay a
    full weight reload. The PE array keeps its weights across matmuls, so
    an LDWEIGHTS identical to the immediately preceding one (and carrying
    no semaphore waits or updates) is a no-op — remove it.
    """
    removed = 0
    for f in nc.m.functions:
        for bb in f.blocks:
            keep = []
            prev_sig = None
            for inst in bb.instructions:
                tn = type(inst).__name__
                if getattr(inst, "engine", None) == mybir.EngineType.PE:
                    if tn == "InstLdweights":
                        sig = str(inst.ins[0]) if inst.ins else None
                        si = inst.sync_info
                        clean = si is None or (
                            len(si.on_wait) == 0 and len(si.on_update) == 0
                        )
                        if sig is not None and sig == prev_sig and clean:
                            removed += 1
                            continue
                        prev_sig = sig
                    elif tn in ("InstMatmult", "InstEventSemaphore", "InstNoOp"):
                        pass  # these don't disturb the loaded weights
                    else:
                        prev_sig = None
                keep.append(inst)
            bb.instructions[:] = keep
    return removed


def _build_nc():
    import concourse.bacc as bacc
    import concourse.tile as tile
    from concourse import mybir

    f32 = mybir.dt.float32
    f16 = mybir.dt.float16
    SIG = mybir.ActivationFunctionType.Sigmoid
    TANH = mybir.ActivationFunctionType.Tanh

    nc = bacc.Bacc("TRN2", target_bir_lowering=False, debug=False)

    at_d = nc.dram_tensor("at", [4 * IN, BS], f16, kind="ExternalInput")
    w_d = {
        g: nc.dram_tensor(f"w{g}", [4 * IN, H], f16, kind="ExternalInput")
        for g in "rzh"
    }
    b_d = {
        g: nc.dram_tensor(f"b{g}", [128, 8], f32, kind="ExternalInput")
        for g in "rzh"
    }
    out_d = nc.dram_tensor("out_t", [H, BS], f32, kind="ExternalOutput")

    with tile.TileContext(nc) as tc:
        with ExitStack() as ctx:
            pp = ctx.enter_context(tc.tile_pool(name="persist", bufs=1))
            wp = ctx.enter_context(tc.tile_pool(name="wslab", bufs=6))
            rp = ctx.enter_context(tc.tile_pool(name="rtmp", bufs=4))
            hpp = ctx.enter_context(tc.tile_pool(name="hprop", bufs=4))
            op = ctx.enter_context(tc.tile_pool(name="otile", bufs=4))
            psp = ctx.enter_context(tc.tile_pool(name="ps", bufs=8, space="PSUM"))

            at_t = [pp.tile([128, BS], f16, tag=f"at{k}", name=f"at{k}") for k in range(KT)]
            rh_t = [pp.tile([128, BS], f16, tag=f"rh{j}", name=f"rh{j}") for j in range(8)]
            z_t = [
                [pp.tile([128, 512], f16, tag=f"z{j}_{b}", name=f"z{j}_{b}") for b in range(NB)]
                for j in range(8)
            ]
            bias_t = {g: pp.tile([128, 8], f32, tag=f"bias{g}", name=f"bias{g}") for g in "rzh"}

            at_dma = [None] * KT

            def ensure_at(k):
                if at_dma[k] is None:
                    at_dma[k] = nc.sync.dma_start(
                        at_t[k][:], at_d.ap()[k * 128:(k + 1) * 128, :]
                    )
                return at_dma[k]

            bias_loaded = [False]

            def ensure_bias():
                # biases are first needed at the first epilogue (~30us in);
                # keep them out of the critical DMA prefix
                if not bias_loaded[0]:
                    for g in "rzh":
                        nc.sync.dma_start(bias_t[g][:], b_d[g].ap()[:, :])
                    bias_loaded[0] = True

            def moving(g, k, b):
                # h-gate contracts k-tiles 8..15 against r*hprev, not hprev
                if g == "h" and 8 <= k < 16:
                    return rh_t[k - 8][:, b * 512:(b + 1) * 512]
                return at_t[k][:, b * 512:(b + 1) * 512]

            def do_group(g, jq):
                # output tiles: j in [jq*256, (jq+1)*256), all BS batch cols
                ps = {}
                for jl in range(2):
                    for b in range(NB):
                        ps[(jl, b)] = psp.tile([128, 512], f32, tag="ps",
                                               name=f"ps_{g}_{jq}_{jl}_{b}")
                for ks in range(KT // KSLAB):
                    k0 = KSLAB * ks
                    slab = wp.tile([128, KSLAB, 256], f16, tag="w", name=f"w_{g}_{jq}_{ks}")
                    src = w_d[g].ap()[k0 * 128:(k0 + KSLAB) * 128,
                                      jq * 256:(jq + 1) * 256]
                    nc.sync.dma_start(slab[:], src.rearrange("(a p) j -> p a j", p=128))
                    for dk in range(KSLAB):
                        k = k0 + dk
                        if not (g == "h" and 8 <= k < 16):
                            ensure_at(k)
                        for jl in range(2):
                            lhsT = slab[:, dk, jl * 128:(jl + 1) * 128]
                            for b in range(NB):
                                nc.tensor.matmul(
                                    ps[(jl, b)][:],
                                    lhsT,
                                    moving(g, k, b),
                                    start=(k == 0),
                                    stop=(k == KT - 1),
                                )
                ensure_bias()
                for jl in range(2):
                    jt = 2 * jq + jl
                    for b in range(NB):
                        pst = ps[(jl, b)]
                        bias_ap = bias_t[g][:, jt:jt + 1]
                        bsl = slice(b * 512, (b + 1) * 512)
                        if g == "r":
                            tmp = rp.tile([128, 512], f32, tag="rt", name=f"rt_{jt}_{b}")
                            nc.scalar.activation(tmp[:], pst[:], SIG, bias=bias_ap)
                            nc.vector.tensor_mul(
                                rh_t[jt][:, bsl], tmp[:], at_t[8 + jt][:, bsl]
                            )
                        elif g == "z":
                            nc.scalar.activation(z_t[jt][b][:], pst[:], SIG, bias=bias_ap)
                        else:
                            hp = hpp.tile([128, 512], f32, tag="hp", name=f"hp_{jt}_{b}")
                            nc.scalar.activation(hp[:], pst[:], TANH, bias=bias_ap)
                            hT = at_t[8 + jt][:, bsl]
                            ot = op.tile([128, 512], f32, tag="ot", name=f"ot_{jt}_{b}")
                            # out = h + z*(hp - h)
                            nc.vector.tensor_sub(ot[:], hp[:], hT)
                            nc.vector.tensor_mul(hp[:], z_t[jt][b][:], ot[:])
                            nc.vector.tensor_add(ot[:], hp[:], hT)
                            nc.sync.dma_start(
                                out_d.ap()[jt * 128:(jt + 1) * 128, bsl], ot[:]
                            )

            for g in ("r", "z", "h"):
                for jq in range(4):
                    do_group(g, jq)

    _dedup_ldweights(nc, mybir)
    nc.finalize()
    return nc


def _get_nc():
    if "nc" not in _CACHE:
        _CACHE["nc"] = _build_nc()
    return _CACHE["nc"]


def _host_prep(inputs):
    x = np.asarray(inputs["x"], dtype=np.float32)
    hprev = np.asarray(inputs["hprev"], dtype=np.float32)
    c = np.asarray(inputs["c"], dtype=np.float32)
    A = np.concatenate([x, hprev, c], axis=1)                  # [B, 4096]
    AT = np.ascontiguousarray(A.T.astype(np.float16))          # [4096, B]
    w = {}
    for g, (W, U, C) in {
        "r": (inputs["Wr"], inputs["Ur"], inputs["Cr"]),
        "z": (inputs["Wz"], inputs["Uz"], inputs["Cz"]),
        "h": (inputs["Wh"], inputs["Uh"], inputs["Ch"]),
    }.items():
        M = np.concatenate(
            [np.asarray(W, np.float32), np.asarray(U, np.float32),
             np.asarray(C, np.float32)],
            axis=1,
        )
        w[g] = np.ascontiguousarray(M.T.astype(np.float16))    # [4096, 1024]
    bias = {
        g: np.ascontiguousarray(
            np.asarray(inputs["b" + g], dtype=np.float32).reshape(8, 128).T
        )
        for g in "rzh"
    }
    return AT, w, bias


def _in_maps(inputs):
    AT, w, bias = _host_prep(inputs)
    maps = []
    for s in range(NCORES):
        maps.append(
            {
                "at": np.ascontiguousarray(AT[:, s * BS:(s + 1) * BS]),
                "wr": w["r"],
                "wz": w["z"],
                "wh": w["h"],
                "br": bias["r"],
                "bz": bias["z"],
                "bh": bias["h"],
            }
        )
    return maps


def run_device(inputs, trace=False, **kwargs):
    """Run the SPMD kernel; returns (full_output, BassKernelResults)."""
    from concourse.bass_utils import run_bass_kernel_spmd

    nc = _get_nc()
    res = run_bass_kernel_spmd(
        nc, _in_maps(inputs), core_ids=list(range(NCORES)), trace=trace, **kwargs
    )
    out = np.empty((B, H), dtype=np.float32)
    for s in range(NCORES):
        out[s * BS:(s + 1) * BS, :] = res.results[s]["out_t"].T
    return out, res


def kernel(**inputs):
    out, _ = run_device(inputs, trace=False)
    return out
